# revision 40
# baseline (speedup 1.0000x reference)
"""CrossAttention Trainium2 kernel (8-core SPMD, batch x head-group sharded).

Problem (hardcoded): x (2,2048,1024) fp32, context (2,2048,1152) fp32,
Wq (1024,1024), Wk/Wv (1024,1152), Wo (1024,1024), zero biases.
16 heads x 64 dim, RoPE (interleaved rotate_half, cat-table), softmax over K,
out projection. Output (2, 2048, 1024) fp32.

Sharding: core c in 0..7 handles batch b = c//4 and head group g = c%4
(heads 4g..4g+3). Each core computes a partial y_c = attn(heads) @ Wo_slice;
host sums 4 partials per batch and adds bo.

v3 design (on top of the v2 software-pipelined schedule):
  - Q/K/V projections in fp8 (e4m3) with the 3-term split trick:
    x = xh + xl, W*64 = Wh + Wl (hi/lo fp8 decomposition, host-side);
    x@W ~= xh@Wh + (xh@Wl + xl@Wh), each term via DoubleRow matmuls that
    process 256-deep contraction at 0.5 cycles/row -> 0.75x the bf16 PE
    cost with ~2x BETTER accuracy than bf16 (hi+lo ~ 9+ mantissa bits).
    The x64 weight scale keeps W (sigma~1/32) out of fp8 denormals; it is
    compensated by exp scale 0.125/4096 and a 64.0 ones-column in v_aug.
  - Per-kt emission order [AV(prev), fillers, scores] so a scores matmul
    stalled on its PSUM bank (exp pacing) never blocks ready work behind
    it in the in-order PE queue.
  - DMA order tuned so the first q/k projections + first exp fire as
    early as the serial DMA track allows; warmup sized to cover the x
    window-0 arrival so the PE p-state ramp isn't reset mid-start.
  - tw0's transpose/O-proj fillers are held back until the late blocks
    (i>=5) where the filler queue otherwise runs dry.
"""
import sys

sys.path.insert(0, "/opt/trn_rl_repo")

import numpy as np
import ml_dtypes
import concourse.bass as bass  # noqa: F401
import concourse.mybir as mybir
import concourse.tile as tile
from concourse import bacc
from concourse.bass_utils import run_bass_kernel_spmd

F32 = mybir.dt.float32
F32R = mybir.dt.float32r
BF16 = mybir.dt.bfloat16
F8 = mybir.dt.float8e4
AF = mybir.ActivationFunctionType
OP = mybir.AluOpType
DR = mybir.MatmulPerfMode.DoubleRow
BF = ml_dtypes.bfloat16
F8NP = ml_dtypes.float8_e4m3

DIM = 1024
SRC = 1152
NH = 16
HD = 64
GH = 4          # heads per core
GD = GH * HD    # 256 projection dims per core
ROPE_BASE = 10000.0
SC = 64.0       # fp8 weight scale (compensated in exp scale + ones col)
WUP = 300       # warmup dummy matmuls (covers DMA wait to x win0)


# ---------------------------------------------------------------- host helpers
def _rope_tables(seq_len: int, head_dim: int):
    inv_freq = 1.0 / (ROPE_BASE ** (np.arange(0, head_dim, 2, dtype=np.float32) / head_dim))
    t = np.arange(seq_len, dtype=np.float32)
    freqs = t[:, None] * inv_freq[None, :]
    emb = np.concatenate([freqs, freqs], axis=-1)  # (L, 64)
    return np.cos(emb).astype(np.float32), np.sin(emb).astype(np.float32)


def _rot128():
    """128x128 rotate_half matrix for a 2-head partition chunk (interleaved)."""
    r64 = np.zeros((HD, HD), dtype=np.float32)
    for i in range(HD // 2):
        r64[2 * i, 2 * i + 1] = -1.0
        r64[2 * i + 1, 2 * i] = 1.0
    return np.kron(np.eye(2, dtype=np.float32), r64)   # (128, 128)


def _hilo_chunks(aT: np.ndarray, nch: int):
    """aT (rows=nch*128, cols) -> fp8 hi/lo [128, 2, nch, cols]; [:,0]=hi."""
    a = np.ascontiguousarray(aT).reshape(nch, 128, -1).transpose(1, 0, 2)
    hi = a.astype(F8NP)
    lo = (a - hi.astype(np.float32)).astype(F8NP)
    return np.ascontiguousarray(np.stack([hi, lo], axis=1))  # [128, 2, nch, cols]


def _hilo_weights(wT: np.ndarray, nch: int):
    """wT (rows=nch*128, cols) scaled weights -> fp8 [128, 2, nch, cols]; [:,0]=lo."""
    a = np.ascontiguousarray(wT).reshape(nch, 128, -1).transpose(1, 0, 2)
    hi = a.astype(F8NP)
    lo = (a - hi.astype(np.float32)).astype(F8NP)
    return np.ascontiguousarray(np.stack([lo, hi], axis=1))  # [128, 2, nch, cols]


# ---------------------------------------------------------------- device build
def build_nc(T: int, K: int, n_cores: int = 8):
    assert T % 1024 == 0 and K % 512 == 0
    NTW = T // 1024         # 1024-wide t windows ("tw")
    NKT = K // 128          # 128-wide k tiles
    NKW = K // 512          # 512-wide k/proj windows
    NQW = T // 512          # 512-wide q proj windows
    NCC = DIM // 128        # x contraction chunks (8)
    NCS = SRC // 128        # context contraction chunks (9)
    EXP_SCALE = 0.125 / (SC * SC)

    nc = bacc.Bacc("TRN2", target_bir_lowering=False, debug=False,
                   num_devices=n_cores)

    x8 = nc.declare_dram_parameter("x8", [128, 2, NCC, T], F8, isOutput=False)
    ct8 = nc.declare_dram_parameter("ct8", [128, 2, NCS, K], F8, isOutput=False)
    wq8 = nc.declare_dram_parameter("wq8", [128, 2, NCC, GD], F8, isOutput=False)
    wk8 = nc.declare_dram_parameter("wk8", [128, 2, NCS, GD], F8, isOutput=False)
    wv8 = nc.declare_dram_parameter("wv8", [128, 2, NCS, GH * 65], F8, isOutput=False)
    woT = nc.declare_dram_parameter("woT", [GD, DIM], BF16, isOutput=False)
    rT = nc.declare_dram_parameter("rT", [128, 128], F32R, isOutput=False)
    csT = nc.declare_dram_parameter("csT", [128, 2, max(T, K)], BF16, isOutput=False)
    idn = nc.declare_dram_parameter("idn", [128, 128], BF16, isOutput=False)
    y = nc.declare_dram_parameter("y", [T, DIM], BF16, isOutput=True)

    with tile.TileContext(nc) as tc:
        with (
            tc.tile_pool(name="consts", bufs=1) as consts,
            tc.tile_pool(name="resid", bufs=1) as resid,
            tc.tile_pool(name="rope", bufs=2) as rope,
            tc.tile_pool(name="wtp", bufs=20) as wtp,
            tc.tile_pool(name="osbp", bufs=1) as osbp,
            tc.tile_pool(name="zp", bufs=4) as zp,
            tc.tile_pool(name="yp", bufs=6) as yp,
            tc.tile_pool(name="pp", bufs=2, space="PSUM") as pp,
            tc.tile_pool(name="sp", bufs=2, space="PSUM") as sp,
            tc.tile_pool(name="avp", bufs=1, space="PSUM") as avp,
        ):
            # ---------------- constants (DMA order = arrival order) ---------
            # Serial DMA track gates the pipeline start: order transfers so
            # the first q proj, then first k proj, then first exp fire as
            # early as possible; every merge saves a fixed HWDGE slot.
            rT_sb = consts.tile([128, 128], F32R, tag="rT")
            nc.sync.dma_start(out=rT_sb, in_=rT[:, :])
            wk_sb = consts.tile([128, 2, NCS, GD], F8, tag="wk")
            nc.sync.dma_start(out=wk_sb, in_=wk8[:, :, :, :])
            ct_sb = consts.tile([128, 2, NCS, K], F8, tag="ct")

            def ct_win_dma(w):
                nc.sync.dma_start(out=ct_sb[:, :, :, w * 512:(w + 1) * 512],
                                  in_=ct8[:, :, :, w * 512:(w + 1) * 512])

            ct_win_dma(0)
            cs_sb = consts.tile([128, 2, max(T, K)], BF16, tag="cs")
            cos_sb = cs_sb[:, 0, :]
            sin_sb = cs_sb[:, 1, :]
            nc.sync.dma_start(out=cs_sb[:, :, 0:1024], in_=csT[:, :, 0:1024])
            wq_sb = consts.tile([128, 2, NCC, GD], F8, tag="wq")
            nc.sync.dma_start(out=wq_sb, in_=wq8[:, :, :, :])
            x_sb = consts.tile([128, 2, NCC, T], F8, tag="x")

            def x_win_dma(w):
                nc.sync.dma_start(out=x_sb[:, :, :, w * 512:(w + 1) * 512],
                                  in_=x8[:, :, :, w * 512:(w + 1) * 512])

            x_win_dma(0)
            x_win_dma(1)
            ct_win_dma(1)
            nc.sync.dma_start(out=cs_sb[:, :, 1024:max(T, K)], in_=csT[:, :, 1024:max(T, K)])
            wv_sb = consts.tile([128, 2, NCS, GH * 65], F8, tag="wv")
            nc.sync.dma_start(out=wv_sb, in_=wv8[:, :, :, :])
            ct_win_dma(2)
            ct_win_dma(3)
            wo_sb = consts.tile([128, 2, DIM], BF16, tag="wo")
            nc.sync.dma_start(out=wo_sb, in_=woT[:, :].rearrange("(o p) f -> p o f", p=128))
            idn_sb = consts.tile([128, 128], BF16, tag="idn")
            nc.sync.dma_start(out=idn_sb, in_=idn[:, :])
            nc.sync.dma_start(out=x_sb[:, :, :, 1024:T], in_=x8[:, :, :, 1024:T])

            # ---------------- residents ------------------------------------
            kT_sb = [resid.tile([128, K], BF16, tag=f"kT{m}", name=f"kT{m}") for m in range(2)]
            qT_sb = [resid.tile([128, T], BF16, tag=f"qT{m}", name=f"qT{m}") for m in range(2)]
            v_sb = [resid.tile([128, GH * 65], BF16, tag=f"v{kt}", name=f"v{kt}")
                    for kt in range(NKT)]
            on_sb = [resid.tile([128, T], BF16, tag=f"on{m}", name=f"on{m}") for m in range(2)]

            wt = {}     # (h, kt) -> wt tile [128, 1024] for current tw
            osb = {}    # tt -> [128, GD] tile for current tw

            # ---------------- emission helpers -----------------------------
            def rope_combine(dst, raw_sb, n, fast=False):
                """dst[:, n*512:+512] (bf16) = raw*cos + (R raw)*sin."""
                t0 = n * 512
                rps = pp.tile([128, 512], F32, tag="pps", name="rps")
                nc.tensor.matmul(rps, rT_sb, raw_sb, start=True, stop=True)
                t1 = rope.tile([128, 512], F32, tag="t1")
                # t1 is SBUF-only -> GPSIMD, freeing the DVE for t2/add;
                # in the latency-critical prefix it goes on the idle DVE
                eng = nc.vector if fast else nc.gpsimd
                eng.tensor_tensor(t1, raw_sb, cos_sb[:, t0:t0 + 512], OP.mult)
                t2 = rope.tile([128, 512], F32, tag="t2")
                nc.vector.tensor_tensor(t2, rps, sin_sb[:, t0:t0 + 512], OP.mult)
                nc.vector.tensor_tensor(dst[:, t0:t0 + 512], t1, t2, OP.add)

            def proj_k(m, n, fast=False):
                """3-term fp8 split K projection, as ~1-1.5k cycle steps."""
                cell = {}
                w_t = wk_sb[:, :, :, m * 128:(m + 1) * 128]
                in_t = ct_sb[:, :, :, n * 512:(n + 1) * 512]

                def s1():
                    kps = pp.tile([128, 512], F32, tag="pps", name="kps")
                    cell['ps'] = kps
                    for a in range(4):          # hi @ hi, chunk pairs
                        nc.tensor.matmul(kps, w_t[:, 1, 2 * a:2 * a + 2, :],
                                         in_t[:, 0, 2 * a:2 * a + 2, :],
                                         start=(a == 0), stop=False, perf_mode=DR)
                    nc.tensor.matmul(kps, w_t[:, 1, NCS - 1, :],
                                     in_t[:, 0, NCS - 1, :],
                                     start=False, stop=False)

                def s2():
                    for a in range(0, 5):       # cross: cth@Wl + ctl@Wh
                        nc.tensor.matmul(cell['ps'], w_t[:, :, a, :],
                                         in_t[:, :, a, :],
                                         start=False, stop=False, perf_mode=DR)

                def s3():
                    for a in range(5, NCS):
                        nc.tensor.matmul(cell['ps'], w_t[:, :, a, :],
                                         in_t[:, :, a, :],
                                         start=False, stop=(a == NCS - 1), perf_mode=DR)

                def s4a():
                    ksb = rope.tile([128, 512], F32R, tag="ksb")
                    cell['sb'] = ksb
                    nc.vector.tensor_copy(ksb, cell['ps'])

                def s4b():
                    rope_combine(kT_sb[m], cell['sb'], n, fast=fast)

                return [(1540, s1), (1280, s2), (1024, s3), (80, s4a), (620, s4b)]

            def proj_q(m, n, fast=False):
                cell = {}
                w_t = wq_sb[:, :, :, m * 128:(m + 1) * 128]
                in_t = x_sb[:, :, :, n * 512:(n + 1) * 512]

                def s1():
                    qps = pp.tile([128, 512], F32, tag="pps", name="qps")
                    cell['ps'] = qps
                    for a in range(4):
                        nc.tensor.matmul(qps, w_t[:, 1, 2 * a:2 * a + 2, :],
                                         in_t[:, 0, 2 * a:2 * a + 2, :],
                                         start=(a == 0), stop=False, perf_mode=DR)

                def s2():
                    for a in range(0, 4):
                        nc.tensor.matmul(cell['ps'], w_t[:, :, a, :],
                                         in_t[:, :, a, :],
                                         start=False, stop=False, perf_mode=DR)

                def s3():
                    for a in range(4, NCC):
                        nc.tensor.matmul(cell['ps'], w_t[:, :, a, :],
                                         in_t[:, :, a, :],
                                         start=False, stop=(a == NCC - 1), perf_mode=DR)

                def s4a():
                    qsb = rope.tile([128, 512], F32R, tag="ksb", name="qsb")
                    cell['sb'] = qsb
                    nc.vector.tensor_copy(qsb, cell['ps'])

                def s4b():
                    rope_combine(qT_sb[m], cell['sb'], n, fast=fast)

                return [(1024, s1), (1024, s2), (1024, s3), (80, s4a), (620, s4b)]

            def proj_v(kt):
                cell = {}
                cts = ct_sb[:, :, :, kt * 128:(kt + 1) * 128]

                def s1():
                    vps = pp.tile([128, GH * 65], F32, tag="pps", name="vps")
                    cell['ps'] = vps
                    for a in range(4):
                        nc.tensor.matmul(vps, cts[:, 0, 2 * a:2 * a + 2, :],
                                         wv_sb[:, 1, 2 * a:2 * a + 2, :],
                                         start=(a == 0), stop=False, perf_mode=DR)
                    nc.tensor.matmul(vps, cts[:, 0, NCS - 1, :],
                                     wv_sb[:, 1, NCS - 1, :], start=False, stop=False)

                def s2():
                    for a in range(NCS):
                        nc.tensor.matmul(cell['ps'], cts[:, :, a, :],
                                         wv_sb[:, :, a, :],
                                         start=False, stop=(a == NCS - 1), perf_mode=DR)
                    nc.vector.tensor_copy(v_sb[kt], cell['ps'])
                    ones_ap = v_sb[kt].rearrange("p (h e) -> p h e", h=GH)[:, :, 64]
                    nc.vector.memset(ones_ap, SC)   # ones col = SC (scale comp)

                return [(780, s1), (1200, s2)]

            def run_steps(steps):
                for _, fn in steps:
                    fn()

            def emit_scores(tw, h, kt, split=False):
                """One scores matmul pair + exp for (head h, t-window tw, k-tile kt).

                split=True exps each 512-half separately so the first exp can
                fire before the second q window's projection is done."""
                m, off = h // 2, (h % 2) * 64
                sps = sp.tile([128, 1024], F32, tag="sps", name="sps")
                wt_t = wtp.tile([128, 1024], BF16, tag="wt", name="wt")
                for half in range(2):
                    sl = slice(half * 512, (half + 1) * 512)
                    nc.tensor.matmul(
                        sps[:, sl],
                        kT_sb[m][off:off + 64, kt * 128:(kt + 1) * 128],
                        qT_sb[m][off:off + 64, tw * 1024 + half * 512:tw * 1024 + (half + 1) * 512],
                        start=True, stop=True)
                    if split:
                        nc.scalar.activation(wt_t[:, sl], sps[:, sl], AF.Exp, scale=EXP_SCALE)
                if not split:
                    nc.scalar.activation(wt_t, sps, AF.Exp, scale=EXP_SCALE)
                wt[(h, kt)] = wt_t

            def emit_av_kt(h, kt, avA, avB, first=False):
                """Accumulate o[t, 65] for all 8 tts of one kt (JIT per-kt).

                The first kt's tt=0/tt=4 matmuls carry start=True: the PSUM
                zero-region (one full bank, tiles are bank-sized) marks the
                rest pending-zero, so no DVE memset is needed and the PE
                write is the only thing that WAR-waits on the previous
                generation's norm reads."""
                for tt in range(8):
                    at = avA if tt < 4 else avB
                    col = (tt % 4) * 65
                    nc.tensor.matmul(
                        at[:, col:col + 65],
                        wt[(h, kt)][:, tt * 128:(tt + 1) * 128],
                        v_sb[kt][:, h * 65:(h + 1) * 65],
                        start=(first and tt in (0, 4)), stop=(kt == NKT - 1),
                        skip_group_check=True)

            def emit_norm(tw, h, av_tile, tts):
                """osb[tt][:, h-cols] = o / Z for the 4 tts of one AV pass."""
                zsb = zp.tile([128, 4], F32, tag="zsb", name="zsb")
                if len(tts) == 4:
                    # single strided gather of the 4 Z columns (64::65)
                    zc = av_tile[:, 0:260].rearrange("p (t e) -> p t e", e=65)[:, :, 64]
                    nc.vector.tensor_copy(zsb, zc)
                else:
                    for j, tt in enumerate(tts):
                        col = (tt % 4) * 65
                        nc.vector.tensor_copy(zsb[:, j:j + 1],
                                              av_tile[:, col + 64:col + 65])
                rec = zp.tile([128, 4], F32, tag="rec", name="rec")
                nc.vector.reciprocal(rec, zsb)
                for j, tt in enumerate(tts):
                    if (tw, tt) not in osb:
                        ot = osbp.tile([128, GD], BF16, tag=f"osb{tw}_{tt}",
                                       name=f"osb{tw}_{tt}")
                        osb[(tw, tt)] = ot
                    col = (tt % 4) * 65
                    nc.vector.tensor_scalar(
                        osb[(tw, tt)][:, h * 64:(h + 1) * 64],
                        av_tile[:, col:col + 64],
                        rec[:, j:j + 1], None, OP.mult)

            def emit_transp(tw, m, tt, ot, tail=False):
                # transpose psum borrows the short-lived proj/yps bank pair;
                # in the tail it uses the idle scores banks + ACT copies so
                # the DVE queue is not the serializer.
                pool, tag = (sp, "sps") if tail else (pp, "pps")
                tps = pool.tile([128, 128], BF16, tag=tag, name="tps")
                nc.tensor.transpose(tps, ot[:, m * 128:(m + 1) * 128], idn_sb)
                dst = on_sb[m][:, tw * 1024 + tt * 128:tw * 1024 + (tt + 1) * 128]
                if tail:
                    nc.scalar.copy(dst, tps)
                else:
                    nc.vector.tensor_copy(dst, tps)

            tail_rot = [0]

            def emit_oproj(tw, tt, use_act=False):
                """Two ~1.2k-cycle steps; y goes out bf16 in one DMA."""
                gtt = tw * 8 + tt
                cell = {}

                def step(nn):
                    if use_act:
                        # tail: rotate psum through the idle scores/AV banks
                        # for a deep copy pipeline (pp alone is only 2 slots)
                        pool, tag = [(sp, "sps"), (avp, "avA"), (avp, "avB"),
                                     (pp, "pps")][tail_rot[0] % 4]
                        tail_rot[0] += 1
                        yps = pool.tile([128, 512], F32, tag=tag, name="yps")
                    else:
                        yps = pp.tile([128, 512], F32, tag="pps", name="yps")
                    for dc in range(2):
                        nc.tensor.matmul(yps, on_sb[dc][:, gtt * 128:(gtt + 1) * 128],
                                         wo_sb[:, dc, nn * 512:(nn + 1) * 512],
                                         start=(dc == 0), stop=(dc == 1))
                    if nn == 0:
                        cell['yt'] = yp.tile([128, 1024], BF16, tag="yt", name="yt")
                    yt = cell['yt']
                    if use_act and tail_rot[0] % 2 == 0:
                        # alternate copy engine so neither serializes the tail
                        nc.scalar.copy(yt[:, nn * 512:(nn + 1) * 512], yps)
                    else:
                        nc.vector.tensor_copy(yt[:, nn * 512:(nn + 1) * 512], yps)
                    if nn == 1:
                        nc.sync.dma_start(out=y[gtt * 128:(gtt + 1) * 128, :], in_=yt)

                return [(1160, lambda: step(0)), (1160, lambda: step(1))]

            # ---------------- filler queue ----------------------------------
            # Deferred PE work in ~1-2k cycle steps, consumed in FIFO order
            # with a cycle budget. Fine steps keep the exp stream fed (a
            # chunky filler would hold up the next scores for >1 exp time).
            # ensure() force-drains through a key so emission order always
            # places producers before consumers (tile deps follow emission
            # order, so this is a correctness requirement, not just pacing).
            fillers = []          # list of (g_min, key|None, cost_cycles, fn)
            held = []             # fillers released late (tw0 O work)
            done_keys = set()
            debt = [0.0]
            giter = [0]           # global iteration counter (i*NKT + kt)

            def add_filler(key, steps, q=None, g=0):
                # g = earliest global iteration this work may be emitted:
                # a filler emitted before its DMA inputs land stalls the
                # whole in-order PE queue, so gates track arrival order.
                q = fillers if q is None else q
                for j, (cost, fn) in enumerate(steps):
                    q.append((g, key if j == len(steps) - 1 else None, cost, fn))

            def pop_filler(charge=True):
                g, key, cost, fn = fillers.pop(0)
                fn()
                if key is not None:
                    done_keys.add(key)
                if charge:
                    debt[0] -= cost

            def emit_fillers(budget):
                # cap so long filler droughts don't bank unbounded credit
                debt[0] = min(debt[0] + budget, 3500.0)
                while (fillers and fillers[0][0] <= giter[0]
                       and fillers[0][2] <= debt[0]):
                    pop_filler()

            def ensure(keys):
                # forced pops are real PE time already spent out-of-budget;
                # don't let them starve later budget-based draining. Gates
                # are ignored: a forced key is needed NOW.
                missing = [k for k in keys if k not in done_keys]
                while missing:
                    if not fillers:
                        raise AssertionError(f"unsatisfiable deps: {missing}")
                    pop_filler(charge=False)
                    missing = [k for k in missing if k not in done_keys]

            def q_keys(m, tw):
                return [("Q", m, n) for n in range(2 * tw, 2 * tw + 2)]

            def scores_guarded(tw, h, kt, split=False):
                m = h // 2
                ensure([("K", m, kt // 4)])
                ensure(q_keys(m, tw))
                emit_scores(tw, h, kt, split=split)

            # ---------------- schedule --------------------------------------
            # PE warm-up: tiny dummy matmuls keep the clock-ramp model hot
            # while the first DMAs land, so the first projections run at
            # full rate (an idle gap resets the p-state ramp). Bursts are
            # interleaved with the prefix to bridge each DMA wait.
            def warm(n):
                wup = pp.tile([8, 8], F32, tag="pps", name="wup")
                for _ in range(n):
                    nc.tensor.matmul(wup, rT_sb[0:64, 0:8], rT_sb[0:64, 0:8],
                                     start=True, stop=True)

            # Prefix in DMA-arrival order: q(0,0) [x w0], k(0,0) [ct w0],
            # q(0,1) [x w1]; warmup bridges the DMA waits in between
            # (counts tuned to each transfer's arrival).
            warm(325)
            k00 = proj_k(0, 0)
            run_steps(k00[:4])       # kps matmuls + psum->sbuf copy
            warm(40)
            run_steps(k00[4:])       # k rope (t1 on Pool, t2/add DVE)
            done_keys.add(("K", 0, 0))
            warm(120)
            run_steps(proj_q(0, 0, fast=True))
            done_keys.add(("Q", 0, 0))
            warm(100)
            q01 = proj_q(0, 1)
            run_steps(q01[:4])
            run_steps(q01[4:])
            done_keys.add(("Q", 0, 1))

            # Static fillers in consumption order, gated by DMA arrival
            # (ct win w lands ~g(3w-2), wv8 ~g5): the budget should drain
            # each before its first use so ensure never forces a PE lump.
            add_filler(("K", 0, 1), proj_k(0, 1), g=1)
            # Q m1/tw0 only needs x (landed): pop during early ct-waits
            add_filler(("Q", 1, 0), proj_q(1, 0), g=2)
            add_filler(("Q", 1, 1), proj_q(1, 1), g=3)
            add_filler(("K", 0, 2), proj_k(0, 2), g=4)
            add_filler(("K", 0, 3), proj_k(0, 3), g=7)
            for kt in range(8):
                add_filler(("V", kt), proj_v(kt), g=5 + kt)
            vg8 = [12, 13, 16, 17, 18, 19, 20, 21]
            for kt in range(8, NKT):
                add_filler(("V", kt), proj_v(kt), g=vg8[kt - 8])
            # K m1 spread over blocks 1-2; tw1 Q projs into blocks 3-4
            # (true deadlines g58 / g90) where the queue otherwise dries up
            kg = [24, 28, 33, 35]
            for n in range(NKW):
                add_filler(("K", 1, n), proj_k(1, n), g=kg[n])
            qg = [44, 46, 60, 64]
            for twl in range(1, NTW):
                for j, m in enumerate((0, 1)):
                    for n in range(twl * 2, twl * 2 + 2):
                        add_filler(("Q", m, n), proj_q(m, n),
                                   g=qg[2 * j + (n - twl * 2)])

            heads = [(tw, h) for tw in range(NTW) for h in range(GH)]

            def head_tail(tw, h, avA, avB):
                """Norm + transpose/oproj bookkeeping after a head's AV."""
                emit_norm(tw, h, avA, [0, 1, 2, 3])
                emit_norm(tw, h, avB, [4, 5, 6, 7])
                last = tw == NTW - 1
                # tw0's O work is held for the late blocks (filler drought);
                # tw1's unlocks late anyway.
                dst = held if tw == 0 else fillers
                if h == 1:
                    for tt in range(8):
                        dst.append((0, ("T", tw, 0, tt), 128,
                                    lambda tw=tw, tt=tt, ot=osb[(tw, tt)]:
                                    emit_transp(tw, 0, tt, ot)))
                elif h == 3:
                    for tt in range(8):
                        dst.append((0, ("T", tw, 1, tt), 128,
                                    lambda tw=tw, tt=tt, ot=osb[(tw, tt)]:
                                    emit_transp(tw, 1, tt, ot)))
                    for tt in range(8):
                        add_filler(("O", tw, tt), emit_oproj(tw, tt, use_act=last),
                                   q=dst)

            # AV for head i runs lagged, inside block i+1 (wt tiles banked),
            # starting at iteration 2: the fresh av tiles' first write (the
            # start=True zeroing matmul) WAR-waits on the previous
            # generation's norm reads, and 2 iterations give the DVE norm
            # chain time to drain so PE/ACT never stall on the transition.
            # Iterations 14/15 process 2 kts each to catch up.
            def av_sched(j):
                if j < 2:
                    return []
                if j < 14:
                    return [j - 2]
                return [2 * j - 16, 2 * j - 15]

            # final block: prev drains at ~3/iter over iters 2..7, prev's
            # norm at iter 8, own-head jit AV (also ~3/iter) over iters
            # 10..15 with the last pair right after the last exp.
            pat6 = [(0, 1, 2), (3, 4, 5), (6, 7, 8), (9, 10, 11), (12, 13), (14, 15)]
            pat6o = [(0, 1, 2), (3, 4, 5), (6, 7, 8), (9, 10, 11), (12, 13, 14), (15,)]

            def new_av():
                a = avp.tile([128, 512], F32, tag="avA", name="avA")
                b = avp.tile([128, 512], F32, tag="avB", name="avB")
                return a, b

            nheads = len(heads)
            avA = avB = None
            for i, (tw, h) in enumerate(heads):
                prev = heads[i - 1] if i > 0 else None
                final = i == nheads - 1
                if i == 5:
                    fillers.extend(held)
                    held.clear()
                for kt in range(NKT):
                    giter[0] = i * NKT + kt
                    # AV + fillers BEFORE scores: a scores matmul stalled on
                    # its psum bank (exp pacing) must not block ready work.
                    if prev is not None and not final:
                        if kt == 2:
                            avA, avB = new_av()
                        for akt in av_sched(kt):
                            ensure([("V", akt)])
                            emit_av_kt(prev[1], akt, avA, avB, first=(akt == 0))
                    if final and 2 <= kt < 8:
                        if kt == 2:
                            avA, avB = new_av()
                        for akt in pat6[kt - 2]:
                            emit_av_kt(prev[1], akt, avA, avB, first=(akt == 0))
                    if final and kt == 8:
                        head_tail(prev[0], prev[1], avA, avB)
                    if kt == 10 and i + 1 < len(heads):
                        ensure(q_keys(heads[i + 1][1] // 2, heads[i + 1][0]))
                    g = giter[0]
                    emit_fillers(3000 if g < 12 else
                                 1500 if g < 16 else
                                 1050 if g < 112 else 300)
                    scores_guarded(tw, h, kt,
                                   split=(i == 0 and kt == 0))
                    if final and kt >= 10:
                        # own-head jit AV comes after this iteration's scores
                        # (the last pair needs wt[(h,15)])
                        if kt == 10:
                            avA, avB = new_av()
                        for akt in pat6[kt - 10]:
                            emit_av_kt(h, akt, avA, avB, first=(akt == 0))
                if prev is not None and not final:
                    head_tail(prev[0], prev[1], avA, avB)

            # tail: lag-1 pipeline - transpose(tt+1) is issued before
            # oproj(tt) so the transpose's ACT copy hides under oproj
            # matmuls, and the first oproj starts right after norm(A).
            tw, h = heads[-1]
            while fillers:
                pop_filler()
            emit_norm(tw, h, avA, [0, 1, 2, 3])
            emit_transp(tw, 1, 0, osb[(tw, 0)], tail=True)
            for tt in range(8):
                if tt == 3:
                    emit_norm(tw, h, avB, [4, 5, 6, 7])
                if tt < 7:
                    emit_transp(tw, 1, tt + 1, osb[(tw, tt + 1)], tail=True)
                run_steps(emit_oproj(tw, tt, use_act=True))

            while fillers:
                pop_filler()

    nc.compile()
    return nc


# ---------------------------------------------------------------- host wrapper
def make_in_maps(x, context, Wq, Wk, Wv, Wo, n_cores=8):
    B, T, _ = x.shape
    K = context.shape[1]
    cos, sin = _rope_tables(max(T, K), HD)      # (L, 64)
    cosT = np.tile(cos.T, (2, 1))               # (128, L)
    sinT = np.tile(sin.T, (2, 1))
    csT = np.ascontiguousarray(np.stack([cosT, sinT], axis=1))   # (128, 2, L)
    rt = np.ascontiguousarray(_rot128().T)

    in_maps = []
    x8b = [_hilo_chunks(x[b].T, DIM // 128) for b in range(B)]
    ct8b = [_hilo_chunks(context[b].T, SRC // 128) for b in range(B)]
    for c in range(n_cores):
        b, g = c // 4, c % 4
        sl = slice(g * GD, (g + 1) * GD)
        wvTa = np.zeros((SRC, GH * 65), dtype=np.float32)
        for h in range(GH):
            wvTa[:, h * 65:h * 65 + 64] = Wv[g * GD + h * HD: g * GD + (h + 1) * HD, :].T
        in_maps.append({
            "x8": x8b[b],
            "ct8": ct8b[b],
            "wq8": _hilo_weights(Wq[sl, :].T * SC, DIM // 128),
            "wk8": _hilo_weights(Wk[sl, :].T * SC, SRC // 128),
            "wv8": _hilo_weights(wvTa * SC, SRC // 128),
            "woT": np.ascontiguousarray(Wo[:, sl].T).astype(BF),
            "rT": rt,
            "csT": csT.astype(BF),
            "idn": np.eye(128, dtype=np.float32).astype(BF),
        })
    return in_maps


def run(nc, in_maps, n_cores=8):
    res = run_bass_kernel_spmd(nc, in_maps, core_ids=list(range(n_cores)))
    return res.results


def kernel(x, context, Wq, bq, Wk, bk, Wv, bv, Wo, bo):
    B, T, _ = x.shape
    K = context.shape[1]
    x = np.asarray(x, dtype=np.float32)
    context = np.asarray(context, dtype=np.float32)
    Wq, Wk, Wv, Wo = (np.asarray(a, dtype=np.float32) for a in (Wq, Wk, Wv, Wo))
    bq, bk, bv, bo = (np.asarray(a, dtype=np.float32) for a in (bq, bk, bv, bo))

    nc = build_nc(T, K, n_cores=8)
    in_maps = make_in_maps(x, context, Wq, Wk, Wv, Wo)
    assert not bq.any() and not bk.any() and not bv.any(), "nonzero qkv bias unsupported"
    results = run(nc, in_maps)

    out = np.zeros((B, T, DIM), dtype=np.float32)
    for c in range(8):
        out[c // 4] += results[c]["y"].astype(np.float32)
    out += bo[None, None, :]
    return out


if __name__ == "__main__":
    rng = np.random.default_rng(0)
    T = K = 2048
    x = rng.standard_normal((2, T, DIM), dtype=np.float32)
    ctx = rng.standard_normal((2, K, SRC), dtype=np.float32)
    Wq = rng.standard_normal((DIM, DIM), dtype=np.float32) / 32
    Wk = rng.standard_normal((DIM, SRC), dtype=np.float32) / 34
    Wv = rng.standard_normal((DIM, SRC), dtype=np.float32) / 34
    Wo = rng.standard_normal((DIM, DIM), dtype=np.float32) / 32
    z = np.zeros(DIM, dtype=np.float32)
    got = kernel(x, ctx, Wq, z, Wk, z, Wv, z, Wo, z)

    def ref(x, ctx):
        q = x @ Wq.T
        k = ctx @ Wk.T
        v = ctx @ Wv.T
        B = x.shape[0]
        q = q.reshape(B, T, NH, HD).transpose(0, 2, 1, 3)
        k = k.reshape(B, K, NH, HD).transpose(0, 2, 1, 3)
        v = v.reshape(B, K, NH, HD).transpose(0, 2, 1, 3)
        cos, sin = _rope_tables(T, HD)

        def rot_half(t):
            t1, t2 = t[..., ::2], t[..., 1::2]
            return np.stack((-t2, t1), axis=-1).reshape(t.shape)

        q = q * cos[None, None] + rot_half(q) * sin[None, None]
        k = k * cos[None, None] + rot_half(k) * sin[None, None]
        s = np.einsum("bhtd,bhkd->bhtk", q, k) / np.sqrt(HD)
        s = np.exp(s - s.max(-1, keepdims=True))
        w = s / s.sum(-1, keepdims=True)
        o = np.einsum("bhtk,bhkd->bhtd", w, v)
        o = o.transpose(0, 2, 1, 3).reshape(B, T, DIM)
        return o @ Wo.T

    want = ref(x, ctx)
    err = np.abs(got - want).max() / np.abs(want).max()
    print("smoke relerr:", err)


# revision 41
# speedup vs baseline: 1.0054x; 1.0054x over previous
"""CrossAttention Trainium2 kernel (8-core SPMD, batch x head-group sharded).

Problem (hardcoded): x (2,2048,1024) fp32, context (2,2048,1152) fp32,
Wq (1024,1024), Wk/Wv (1024,1152), Wo (1024,1024), zero biases.
16 heads x 64 dim, RoPE (interleaved rotate_half, cat-table), softmax over K,
out projection. Output (2, 2048, 1024) fp32.

Sharding: core c in 0..7 handles batch b = c//4 and head group g = c%4
(heads 4g..4g+3). Each core computes a partial y_c = attn(heads) @ Wo_slice;
host sums 4 partials per batch and adds bo.

v3 design (on top of the v2 software-pipelined schedule):
  - Q/K/V projections in fp8 (e4m3) with the 3-term split trick:
    x = xh + xl, W*64 = Wh + Wl (hi/lo fp8 decomposition, host-side);
    x@W ~= xh@Wh + (xh@Wl + xl@Wh), each term via DoubleRow matmuls that
    process 256-deep contraction at 0.5 cycles/row -> 0.75x the bf16 PE
    cost with ~2x BETTER accuracy than bf16 (hi+lo ~ 9+ mantissa bits).
    The x64 weight scale keeps W (sigma~1/32) out of fp8 denormals; it is
    compensated by exp scale 0.125/4096 and a 64.0 ones-column in v_aug.
  - Per-kt emission order [AV(prev), fillers, scores] so a scores matmul
    stalled on its PSUM bank (exp pacing) never blocks ready work behind
    it in the in-order PE queue.
  - DMA order tuned so the first q/k projections + first exp fire as
    early as the serial DMA track allows; warmup sized to cover the x
    window-0 arrival so the PE p-state ramp isn't reset mid-start.
  - tw0's transpose/O-proj fillers are held back until the late blocks
    (i>=5) where the filler queue otherwise runs dry.
"""
import sys

sys.path.insert(0, "/opt/trn_rl_repo")

import numpy as np
import ml_dtypes
import concourse.bass as bass  # noqa: F401
import concourse.mybir as mybir
import concourse.tile as tile
from concourse import bacc
from concourse.bass_utils import run_bass_kernel_spmd

F32 = mybir.dt.float32
F32R = mybir.dt.float32r
BF16 = mybir.dt.bfloat16
F8 = mybir.dt.float8e4
AF = mybir.ActivationFunctionType
OP = mybir.AluOpType
DR = mybir.MatmulPerfMode.DoubleRow
BF = ml_dtypes.bfloat16
F8NP = ml_dtypes.float8_e4m3

DIM = 1024
SRC = 1152
NH = 16
HD = 64
GH = 4          # heads per core
GD = GH * HD    # 256 projection dims per core
ROPE_BASE = 10000.0
SC = 64.0       # fp8 weight scale (compensated in exp scale + ones col)
WUP = 300       # warmup dummy matmuls (covers DMA wait to x win0)


# ---------------------------------------------------------------- host helpers
def _rope_tables(seq_len: int, head_dim: int):
    inv_freq = 1.0 / (ROPE_BASE ** (np.arange(0, head_dim, 2, dtype=np.float32) / head_dim))
    t = np.arange(seq_len, dtype=np.float32)
    freqs = t[:, None] * inv_freq[None, :]
    emb = np.concatenate([freqs, freqs], axis=-1)  # (L, 64)
    return np.cos(emb).astype(np.float32), np.sin(emb).astype(np.float32)


def _rot128():
    """128x128 rotate_half matrix for a 2-head partition chunk (interleaved)."""
    r64 = np.zeros((HD, HD), dtype=np.float32)
    for i in range(HD // 2):
        r64[2 * i, 2 * i + 1] = -1.0
        r64[2 * i + 1, 2 * i] = 1.0
    return np.kron(np.eye(2, dtype=np.float32), r64)   # (128, 128)


def _hilo_chunks(aT: np.ndarray, nch: int):
    """aT (rows=nch*128, cols) -> fp8 hi/lo [128, 2, nch, cols]; [:,0]=hi."""
    a = np.ascontiguousarray(aT).reshape(nch, 128, -1).transpose(1, 0, 2)
    hi = a.astype(F8NP)
    lo = (a - hi.astype(np.float32)).astype(F8NP)
    return np.ascontiguousarray(np.stack([hi, lo], axis=1))  # [128, 2, nch, cols]


def _hilo_weights(wT: np.ndarray, nch: int):
    """wT (rows=nch*128, cols) scaled weights -> fp8 [128, 2, nch, cols]; [:,0]=lo."""
    a = np.ascontiguousarray(wT).reshape(nch, 128, -1).transpose(1, 0, 2)
    hi = a.astype(F8NP)
    lo = (a - hi.astype(np.float32)).astype(F8NP)
    return np.ascontiguousarray(np.stack([lo, hi], axis=1))  # [128, 2, nch, cols]


# ---------------------------------------------------------------- device build
def build_nc(T: int, K: int, n_cores: int = 8):
    assert T % 1024 == 0 and K % 512 == 0
    NTW = T // 1024         # 1024-wide t windows ("tw")
    NKT = K // 128          # 128-wide k tiles
    NKW = K // 512          # 512-wide k/proj windows
    NQW = T // 512          # 512-wide q proj windows
    NCC = DIM // 128        # x contraction chunks (8)
    NCS = SRC // 128        # context contraction chunks (9)
    EXP_SCALE = 0.125 / (SC * SC)

    nc = bacc.Bacc("TRN2", target_bir_lowering=False, debug=False,
                   num_devices=n_cores)

    x8 = nc.declare_dram_parameter("x8", [128, 2, NCC, T], F8, isOutput=False)
    ct8 = nc.declare_dram_parameter("ct8", [128, 2, NCS, K], F8, isOutput=False)
    wq8 = nc.declare_dram_parameter("wq8", [128, 2, NCC, GD], F8, isOutput=False)
    wk8 = nc.declare_dram_parameter("wk8", [128, 2, NCS, GD], F8, isOutput=False)
    wv8 = nc.declare_dram_parameter("wv8", [128, 2, NCS, GH * 65], F8, isOutput=False)
    woT = nc.declare_dram_parameter("woT", [GD, DIM], BF16, isOutput=False)
    rT = nc.declare_dram_parameter("rT", [128, 128], F32R, isOutput=False)
    csT = nc.declare_dram_parameter("csT", [128, 2, max(T, K)], BF16, isOutput=False)
    idn = nc.declare_dram_parameter("idn", [128, 128], BF16, isOutput=False)
    y = nc.declare_dram_parameter("y", [T, DIM], BF16, isOutput=True)

    with tile.TileContext(nc) as tc:
        with (
            tc.tile_pool(name="consts", bufs=1) as consts,
            tc.tile_pool(name="resid", bufs=1) as resid,
            tc.tile_pool(name="rope", bufs=2) as rope,
            tc.tile_pool(name="wtp", bufs=20) as wtp,
            tc.tile_pool(name="osbp", bufs=1) as osbp,
            tc.tile_pool(name="zp", bufs=4) as zp,
            tc.tile_pool(name="yp", bufs=6) as yp,
            tc.tile_pool(name="pp", bufs=2, space="PSUM") as pp,
            tc.tile_pool(name="sp", bufs=2, space="PSUM") as sp,
            tc.tile_pool(name="avp", bufs=1, space="PSUM") as avp,
        ):
            # ---------------- constants (DMA order = arrival order) ---------
            # Serial DMA track gates the pipeline start: order transfers so
            # the first q proj, then first k proj, then first exp fire as
            # early as possible; every merge saves a fixed HWDGE slot.
            rT_sb = consts.tile([128, 128], F32R, tag="rT")
            nc.sync.dma_start(out=rT_sb, in_=rT[:, :])
            wk_sb = consts.tile([128, 2, NCS, GD], F8, tag="wk")
            nc.sync.dma_start(out=wk_sb, in_=wk8[:, :, :, :])
            ct_sb = consts.tile([128, 2, NCS, K], F8, tag="ct")

            def ct_win_dma(w):
                nc.sync.dma_start(out=ct_sb[:, :, :, w * 512:(w + 1) * 512],
                                  in_=ct8[:, :, :, w * 512:(w + 1) * 512])

            ct_win_dma(0)
            cs_sb = consts.tile([128, 2, max(T, K)], BF16, tag="cs")
            cos_sb = cs_sb[:, 0, :]
            sin_sb = cs_sb[:, 1, :]
            nc.sync.dma_start(out=cs_sb[:, :, 0:1024], in_=csT[:, :, 0:1024])
            wq_sb = consts.tile([128, 2, NCC, GD], F8, tag="wq")
            nc.sync.dma_start(out=wq_sb, in_=wq8[:, :, :, :])
            x_sb = consts.tile([128, 2, NCC, T], F8, tag="x")

            def x_win_dma(w):
                nc.sync.dma_start(out=x_sb[:, :, :, w * 512:(w + 1) * 512],
                                  in_=x8[:, :, :, w * 512:(w + 1) * 512])

            x_win_dma(0)
            x_win_dma(1)
            ct_win_dma(1)
            nc.sync.dma_start(out=cs_sb[:, :, 1024:max(T, K)], in_=csT[:, :, 1024:max(T, K)])
            wv_sb = consts.tile([128, 2, NCS, GH * 65], F8, tag="wv")
            nc.sync.dma_start(out=wv_sb, in_=wv8[:, :, :, :])
            ct_win_dma(2)
            ct_win_dma(3)
            wo_sb = consts.tile([128, 2, DIM], BF16, tag="wo")
            nc.sync.dma_start(out=wo_sb, in_=woT[:, :].rearrange("(o p) f -> p o f", p=128))
            idn_sb = consts.tile([128, 128], BF16, tag="idn")
            nc.sync.dma_start(out=idn_sb, in_=idn[:, :])
            nc.sync.dma_start(out=x_sb[:, :, :, 1024:T], in_=x8[:, :, :, 1024:T])

            # ---------------- residents ------------------------------------
            kT_sb = [resid.tile([128, K], BF16, tag=f"kT{m}", name=f"kT{m}") for m in range(2)]
            qT_sb = [resid.tile([128, T], BF16, tag=f"qT{m}", name=f"qT{m}") for m in range(2)]
            v_sb = [resid.tile([128, GH * 65], BF16, tag=f"v{kt}", name=f"v{kt}")
                    for kt in range(NKT)]
            on_sb = [resid.tile([128, T], BF16, tag=f"on{m}", name=f"on{m}") for m in range(2)]

            wt = {}     # (h, kt) -> wt tile [128, 1024] for current tw
            osb = {}    # tt -> [128, GD] tile for current tw

            # ---------------- emission helpers -----------------------------
            def rope_combine(dst, raw_sb, n, fast=False):
                """dst[:, n*512:+512] (bf16) = raw*cos + (R raw)*sin."""
                t0 = n * 512
                rps = pp.tile([128, 512], F32, tag="pps", name="rps")
                nc.tensor.matmul(rps, rT_sb, raw_sb, start=True, stop=True)
                t1 = rope.tile([128, 512], F32, tag="t1")
                # t1 is SBUF-only -> GPSIMD, freeing the DVE for t2/add;
                # in the latency-critical prefix it goes on the idle DVE
                eng = nc.vector if fast else nc.gpsimd
                eng.tensor_tensor(t1, raw_sb, cos_sb[:, t0:t0 + 512], OP.mult)
                t2 = rope.tile([128, 512], F32, tag="t2")
                nc.vector.tensor_tensor(t2, rps, sin_sb[:, t0:t0 + 512], OP.mult)
                nc.vector.tensor_tensor(dst[:, t0:t0 + 512], t1, t2, OP.add)

            def proj_k(m, n, fast=False):
                """3-term fp8 split K projection, as ~1-1.5k cycle steps."""
                cell = {}
                w_t = wk_sb[:, :, :, m * 128:(m + 1) * 128]
                in_t = ct_sb[:, :, :, n * 512:(n + 1) * 512]

                def s1():
                    kps = pp.tile([128, 512], F32, tag="pps", name="kps")
                    cell['ps'] = kps
                    for a in range(4):          # hi @ hi, chunk pairs
                        nc.tensor.matmul(kps, w_t[:, 1, 2 * a:2 * a + 2, :],
                                         in_t[:, 0, 2 * a:2 * a + 2, :],
                                         start=(a == 0), stop=False, perf_mode=DR)
                    nc.tensor.matmul(kps, w_t[:, 1, NCS - 1, :],
                                     in_t[:, 0, NCS - 1, :],
                                     start=False, stop=False)

                def s2():
                    for a in range(0, 5):       # cross: cth@Wl + ctl@Wh
                        nc.tensor.matmul(cell['ps'], w_t[:, :, a, :],
                                         in_t[:, :, a, :],
                                         start=False, stop=False, perf_mode=DR)

                def s3():
                    for a in range(5, NCS):
                        nc.tensor.matmul(cell['ps'], w_t[:, :, a, :],
                                         in_t[:, :, a, :],
                                         start=False, stop=(a == NCS - 1), perf_mode=DR)

                def s4a():
                    ksb = rope.tile([128, 512], F32R, tag="ksb")
                    cell['sb'] = ksb
                    nc.vector.tensor_copy(ksb, cell['ps'])

                def s4b():
                    rope_combine(kT_sb[m], cell['sb'], n, fast=fast)

                return [(1540, s1), (1280, s2), (1024, s3), (80, s4a), (620, s4b)]

            def proj_q(m, n, fast=False):
                cell = {}
                w_t = wq_sb[:, :, :, m * 128:(m + 1) * 128]
                in_t = x_sb[:, :, :, n * 512:(n + 1) * 512]

                def s1():
                    qps = pp.tile([128, 512], F32, tag="pps", name="qps")
                    cell['ps'] = qps
                    for a in range(4):
                        nc.tensor.matmul(qps, w_t[:, 1, 2 * a:2 * a + 2, :],
                                         in_t[:, 0, 2 * a:2 * a + 2, :],
                                         start=(a == 0), stop=False, perf_mode=DR)

                def s2():
                    for a in range(0, 4):
                        nc.tensor.matmul(cell['ps'], w_t[:, :, a, :],
                                         in_t[:, :, a, :],
                                         start=False, stop=False, perf_mode=DR)

                def s3():
                    for a in range(4, NCC):
                        nc.tensor.matmul(cell['ps'], w_t[:, :, a, :],
                                         in_t[:, :, a, :],
                                         start=False, stop=(a == NCC - 1), perf_mode=DR)

                def s4a():
                    qsb = rope.tile([128, 512], F32R, tag="ksb", name="qsb")
                    cell['sb'] = qsb
                    nc.vector.tensor_copy(qsb, cell['ps'])

                def s4b():
                    rope_combine(qT_sb[m], cell['sb'], n, fast=fast)

                return [(1024, s1), (1024, s2), (1024, s3), (80, s4a), (620, s4b)]

            def proj_v(kt):
                cell = {}
                cts = ct_sb[:, :, :, kt * 128:(kt + 1) * 128]

                def s1():
                    vps = pp.tile([128, GH * 65], F32, tag="pps", name="vps")
                    cell['ps'] = vps
                    for a in range(4):
                        nc.tensor.matmul(vps, cts[:, 0, 2 * a:2 * a + 2, :],
                                         wv_sb[:, 1, 2 * a:2 * a + 2, :],
                                         start=(a == 0), stop=False, perf_mode=DR)
                    nc.tensor.matmul(vps, cts[:, 0, NCS - 1, :],
                                     wv_sb[:, 1, NCS - 1, :], start=False, stop=False)

                def s2():
                    for a in range(NCS):
                        nc.tensor.matmul(cell['ps'], cts[:, :, a, :],
                                         wv_sb[:, :, a, :],
                                         start=False, stop=(a == NCS - 1), perf_mode=DR)
                    nc.vector.tensor_copy(v_sb[kt], cell['ps'])
                    ones_ap = v_sb[kt].rearrange("p (h e) -> p h e", h=GH)[:, :, 64]
                    nc.vector.memset(ones_ap, SC)   # ones col = SC (scale comp)

                return [(780, s1), (1200, s2)]

            def run_steps(steps):
                for _, fn in steps:
                    fn()

            def emit_scores(tw, h, kt, split=False):
                """One scores matmul pair + exp for (head h, t-window tw, k-tile kt).

                split=True exps each 512-half separately so the first exp can
                fire before the second q window's projection is done."""
                m, off = h // 2, (h % 2) * 64
                sps = sp.tile([128, 1024], F32, tag="sps", name="sps")
                wt_t = wtp.tile([128, 1024], BF16, tag="wt", name="wt")
                for half in range(2):
                    sl = slice(half * 512, (half + 1) * 512)
                    nc.tensor.matmul(
                        sps[:, sl],
                        kT_sb[m][off:off + 64, kt * 128:(kt + 1) * 128],
                        qT_sb[m][off:off + 64, tw * 1024 + half * 512:tw * 1024 + (half + 1) * 512],
                        start=True, stop=True)
                    if split:
                        nc.scalar.activation(wt_t[:, sl], sps[:, sl], AF.Exp, scale=EXP_SCALE)
                if not split:
                    nc.scalar.activation(wt_t, sps, AF.Exp, scale=EXP_SCALE)
                wt[(h, kt)] = wt_t

            def emit_av_kt(h, kt, avA, avB, first=False):
                """Accumulate o[t, 65] for all 8 tts of one kt (JIT per-kt).

                The first kt's tt=0/tt=4 matmuls carry start=True: the PSUM
                zero-region (one full bank, tiles are bank-sized) marks the
                rest pending-zero, so no DVE memset is needed and the PE
                write is the only thing that WAR-waits on the previous
                generation's norm reads."""
                for tt in range(8):
                    at = avA if tt < 4 else avB
                    col = (tt % 4) * 65
                    nc.tensor.matmul(
                        at[:, col:col + 65],
                        wt[(h, kt)][:, tt * 128:(tt + 1) * 128],
                        v_sb[kt][:, h * 65:(h + 1) * 65],
                        start=(first and tt in (0, 4)), stop=(kt == NKT - 1),
                        skip_group_check=True)

            def emit_norm(tw, h, av_tile, tts):
                """osb[tt][:, h-cols] = o / Z for the 4 tts of one AV pass."""
                zsb = zp.tile([128, 4], F32, tag="zsb", name="zsb")
                for j, tt in enumerate(tts):
                    col = (tt % 4) * 65
                    nc.vector.tensor_copy(zsb[:, j:j + 1],
                                          av_tile[:, col + 64:col + 65])
                rec = zp.tile([128, 4], F32, tag="rec", name="rec")
                nc.vector.reciprocal(rec, zsb)
                for j, tt in enumerate(tts):
                    if (tw, tt) not in osb:
                        ot = osbp.tile([128, GD], BF16, tag=f"osb{tw}_{tt}",
                                       name=f"osb{tw}_{tt}")
                        osb[(tw, tt)] = ot
                    col = (tt % 4) * 65
                    nc.vector.tensor_scalar(
                        osb[(tw, tt)][:, h * 64:(h + 1) * 64],
                        av_tile[:, col:col + 64],
                        rec[:, j:j + 1], None, OP.mult)

            def emit_transp(tw, m, tt, ot, tail=False):
                # transpose psum borrows the short-lived proj/yps bank pair;
                # in the tail it uses the idle scores banks + ACT copies so
                # the DVE queue is not the serializer.
                pool, tag = (sp, "sps") if tail else (pp, "pps")
                tps = pool.tile([128, 128], BF16, tag=tag, name="tps")
                nc.tensor.transpose(tps, ot[:, m * 128:(m + 1) * 128], idn_sb)
                dst = on_sb[m][:, tw * 1024 + tt * 128:tw * 1024 + (tt + 1) * 128]
                if tail:
                    nc.scalar.copy(dst, tps)
                else:
                    nc.vector.tensor_copy(dst, tps)

            tail_rot = [0]

            def emit_oproj(tw, tt, use_act=False):
                """Two ~1.2k-cycle steps; y goes out bf16 in one DMA."""
                gtt = tw * 8 + tt
                cell = {}

                def step(nn):
                    if use_act:
                        # tail: rotate psum through the idle scores/AV banks
                        # for a deep copy pipeline (pp alone is only 2 slots)
                        pool, tag = [(sp, "sps"), (avp, "avA"), (avp, "avB"),
                                     (pp, "pps")][tail_rot[0] % 4]
                        tail_rot[0] += 1
                        yps = pool.tile([128, 512], F32, tag=tag, name="yps")
                    else:
                        yps = pp.tile([128, 512], F32, tag="pps", name="yps")
                    for dc in range(2):
                        nc.tensor.matmul(yps, on_sb[dc][:, gtt * 128:(gtt + 1) * 128],
                                         wo_sb[:, dc, nn * 512:(nn + 1) * 512],
                                         start=(dc == 0), stop=(dc == 1))
                    if nn == 0:
                        cell['yt'] = yp.tile([128, 1024], BF16, tag="yt", name="yt")
                    yt = cell['yt']
                    if use_act and tail_rot[0] % 2 == 0:
                        # alternate copy engine so neither serializes the tail
                        nc.scalar.copy(yt[:, nn * 512:(nn + 1) * 512], yps)
                    else:
                        nc.vector.tensor_copy(yt[:, nn * 512:(nn + 1) * 512], yps)
                    if nn == 1:
                        nc.sync.dma_start(out=y[gtt * 128:(gtt + 1) * 128, :], in_=yt)

                return [(1160, lambda: step(0)), (1160, lambda: step(1))]

            # ---------------- filler queue ----------------------------------
            # Deferred PE work in ~1-2k cycle steps, consumed in FIFO order
            # with a cycle budget. Fine steps keep the exp stream fed (a
            # chunky filler would hold up the next scores for >1 exp time).
            # ensure() force-drains through a key so emission order always
            # places producers before consumers (tile deps follow emission
            # order, so this is a correctness requirement, not just pacing).
            fillers = []          # list of (g_min, key|None, cost_cycles, fn)
            held = []             # fillers released late (tw0 O work)
            done_keys = set()
            debt = [0.0]
            giter = [0]           # global iteration counter (i*NKT + kt)

            def add_filler(key, steps, q=None, g=0):
                # g = earliest global iteration this work may be emitted:
                # a filler emitted before its DMA inputs land stalls the
                # whole in-order PE queue, so gates track arrival order.
                q = fillers if q is None else q
                for j, (cost, fn) in enumerate(steps):
                    q.append((g, key if j == len(steps) - 1 else None, cost, fn))

            def pop_filler(charge=True):
                g, key, cost, fn = fillers.pop(0)
                fn()
                if key is not None:
                    done_keys.add(key)
                if charge:
                    debt[0] -= cost

            def emit_fillers(budget):
                # cap so long filler droughts don't bank unbounded credit
                debt[0] = min(debt[0] + budget, 3500.0)
                while (fillers and fillers[0][0] <= giter[0]
                       and fillers[0][2] <= debt[0]):
                    pop_filler()

            def ensure(keys):
                # forced pops are real PE time already spent out-of-budget;
                # don't let them starve later budget-based draining. Gates
                # are ignored: a forced key is needed NOW.
                missing = [k for k in keys if k not in done_keys]
                while missing:
                    if not fillers:
                        raise AssertionError(f"unsatisfiable deps: {missing}")
                    pop_filler(charge=False)
                    missing = [k for k in missing if k not in done_keys]

            def q_keys(m, tw):
                return [("Q", m, n) for n in range(2 * tw, 2 * tw + 2)]

            def scores_guarded(tw, h, kt, split=False):
                m = h // 2
                ensure([("K", m, kt // 4)])
                ensure(q_keys(m, tw))
                emit_scores(tw, h, kt, split=split)

            # ---------------- schedule --------------------------------------
            # PE warm-up: tiny dummy matmuls keep the clock-ramp model hot
            # while the first DMAs land, so the first projections run at
            # full rate (an idle gap resets the p-state ramp). Bursts are
            # interleaved with the prefix to bridge each DMA wait.
            def warm(n):
                wup = pp.tile([8, 8], F32, tag="pps", name="wup")
                for _ in range(n):
                    nc.tensor.matmul(wup, rT_sb[0:64, 0:8], rT_sb[0:64, 0:8],
                                     start=True, stop=True)

            # Prefix in DMA-arrival order: q(0,0) [x w0], k(0,0) [ct w0],
            # q(0,1) [x w1]; warmup bridges the DMA waits in between
            # (counts tuned to each transfer's arrival).
            warm(325)
            k00 = proj_k(0, 0)
            run_steps(k00[:4])       # kps matmuls + psum->sbuf copy
            warm(40)
            run_steps(k00[4:])       # k rope (t1 on Pool, t2/add DVE)
            done_keys.add(("K", 0, 0))
            warm(120)
            run_steps(proj_q(0, 0, fast=True))
            done_keys.add(("Q", 0, 0))
            warm(100)
            q01 = proj_q(0, 1)
            run_steps(q01[:4])
            run_steps(q01[4:])
            done_keys.add(("Q", 0, 1))

            # Static fillers in consumption order, gated by DMA arrival
            # (ct win w lands ~g(3w-2), wv8 ~g5): the budget should drain
            # each before its first use so ensure never forces a PE lump.
            add_filler(("K", 0, 1), proj_k(0, 1), g=1)
            # Q m1/tw0 only needs x (landed): pop during early ct-waits
            add_filler(("Q", 1, 0), proj_q(1, 0), g=2)
            add_filler(("Q", 1, 1), proj_q(1, 1), g=3)
            add_filler(("K", 0, 2), proj_k(0, 2), g=4)
            add_filler(("K", 0, 3), proj_k(0, 3), g=7)
            for kt in range(8):
                add_filler(("V", kt), proj_v(kt), g=5 + kt)
            vg8 = [12, 13, 16, 17, 18, 19, 20, 21]
            for kt in range(8, NKT):
                add_filler(("V", kt), proj_v(kt), g=vg8[kt - 8])
            # K m1 spread over blocks 1-2; tw1 Q projs into blocks 3-4
            # (true deadlines g58 / g90) where the queue otherwise dries up
            kg = [24, 28, 33, 35]
            for n in range(NKW):
                add_filler(("K", 1, n), proj_k(1, n), g=kg[n])
            qg = [44, 46, 60, 64]
            for twl in range(1, NTW):
                for j, m in enumerate((0, 1)):
                    for n in range(twl * 2, twl * 2 + 2):
                        add_filler(("Q", m, n), proj_q(m, n),
                                   g=qg[2 * j + (n - twl * 2)])

            heads = [(tw, h) for tw in range(NTW) for h in range(GH)]

            def head_tail(tw, h, avA, avB):
                """Norm + transpose/oproj bookkeeping after a head's AV."""
                emit_norm(tw, h, avA, [0, 1, 2, 3])
                emit_norm(tw, h, avB, [4, 5, 6, 7])
                last = tw == NTW - 1
                # tw0's O work is held for the late blocks (filler drought);
                # tw1's unlocks late anyway.
                dst = held if tw == 0 else fillers
                if h == 1:
                    for tt in range(8):
                        dst.append((0, ("T", tw, 0, tt), 128,
                                    lambda tw=tw, tt=tt, ot=osb[(tw, tt)]:
                                    emit_transp(tw, 0, tt, ot)))
                elif h == 3:
                    for tt in range(8):
                        dst.append((0, ("T", tw, 1, tt), 128,
                                    lambda tw=tw, tt=tt, ot=osb[(tw, tt)]:
                                    emit_transp(tw, 1, tt, ot)))
                    for tt in range(8):
                        add_filler(("O", tw, tt), emit_oproj(tw, tt, use_act=last),
                                   q=dst)

            # AV for head i runs lagged, inside block i+1 (wt tiles banked),
            # starting at iteration 2: the fresh av tiles' first write (the
            # start=True zeroing matmul) WAR-waits on the previous
            # generation's norm reads, and 2 iterations give the DVE norm
            # chain time to drain so PE/ACT never stall on the transition.
            # Iterations 14/15 process 2 kts each to catch up.
            def av_sched(j):
                if j < 2:
                    return []
                if j < 14:
                    return [j - 2]
                return [2 * j - 16, 2 * j - 15]

            # final block: prev drains at ~3/iter over iters 2..7, prev's
            # norm at iter 8, own-head jit AV (also ~3/iter) over iters
            # 10..15 with the last pair right after the last exp.
            pat6 = [(0, 1, 2), (3, 4, 5), (6, 7, 8), (9, 10, 11), (12, 13), (14, 15)]
            pat6o = [(0, 1, 2), (3, 4, 5), (6, 7, 8), (9, 10, 11), (12, 13, 14), (15,)]

            def new_av():
                a = avp.tile([128, 512], F32, tag="avA", name="avA")
                b = avp.tile([128, 512], F32, tag="avB", name="avB")
                return a, b

            nheads = len(heads)
            avA = avB = None
            for i, (tw, h) in enumerate(heads):
                prev = heads[i - 1] if i > 0 else None
                final = i == nheads - 1
                if i == 5:
                    fillers.extend(held)
                    held.clear()
                for kt in range(NKT):
                    giter[0] = i * NKT + kt
                    # AV + fillers BEFORE scores: a scores matmul stalled on
                    # its psum bank (exp pacing) must not block ready work.
                    if prev is not None and not final:
                        if kt == 2:
                            avA, avB = new_av()
                        for akt in av_sched(kt):
                            ensure([("V", akt)])
                            emit_av_kt(prev[1], akt, avA, avB, first=(akt == 0))
                    if final and 2 <= kt < 8:
                        if kt == 2:
                            avA, avB = new_av()
                        for akt in pat6[kt - 2]:
                            emit_av_kt(prev[1], akt, avA, avB, first=(akt == 0))
                    if final and kt == 8:
                        head_tail(prev[0], prev[1], avA, avB)
                    if kt == 10 and i + 1 < len(heads):
                        ensure(q_keys(heads[i + 1][1] // 2, heads[i + 1][0]))
                    g = giter[0]
                    emit_fillers(3000 if g < 12 else
                                 1500 if g < 16 else
                                 1050 if g < 112 else 300)
                    scores_guarded(tw, h, kt,
                                   split=(i == 0 and kt == 0))
                    if final and kt >= 10:
                        # own-head jit AV comes after this iteration's scores
                        # (the last pair needs wt[(h,15)])
                        if kt == 10:
                            avA, avB = new_av()
                        for akt in pat6[kt - 10]:
                            emit_av_kt(h, akt, avA, avB, first=(akt == 0))
                if prev is not None and not final:
                    head_tail(prev[0], prev[1], avA, avB)

            # tail: lag-1 pipeline - transpose(tt+1) is issued before
            # oproj(tt) so the transpose's ACT copy hides under oproj
            # matmuls, and the first oproj starts right after norm(A).
            tw, h = heads[-1]
            while fillers:
                pop_filler()
            emit_norm(tw, h, avA, [0, 1, 2, 3])
            emit_transp(tw, 1, 0, osb[(tw, 0)], tail=True)
            for tt in range(8):
                if tt == 3:
                    emit_norm(tw, h, avB, [4, 5, 6, 7])
                if tt < 7:
                    emit_transp(tw, 1, tt + 1, osb[(tw, tt + 1)], tail=True)
                run_steps(emit_oproj(tw, tt, use_act=True))

            while fillers:
                pop_filler()

    nc.compile()
    return nc


# ---------------------------------------------------------------- host wrapper
def make_in_maps(x, context, Wq, Wk, Wv, Wo, n_cores=8):
    B, T, _ = x.shape
    K = context.shape[1]
    cos, sin = _rope_tables(max(T, K), HD)      # (L, 64)
    cosT = np.tile(cos.T, (2, 1))               # (128, L)
    sinT = np.tile(sin.T, (2, 1))
    csT = np.ascontiguousarray(np.stack([cosT, sinT], axis=1))   # (128, 2, L)
    rt = np.ascontiguousarray(_rot128().T)

    in_maps = []
    x8b = [_hilo_chunks(x[b].T, DIM // 128) for b in range(B)]
    ct8b = [_hilo_chunks(context[b].T, SRC // 128) for b in range(B)]
    for c in range(n_cores):
        b, g = c // 4, c % 4
        sl = slice(g * GD, (g + 1) * GD)
        wvTa = np.zeros((SRC, GH * 65), dtype=np.float32)
        for h in range(GH):
            wvTa[:, h * 65:h * 65 + 64] = Wv[g * GD + h * HD: g * GD + (h + 1) * HD, :].T
        in_maps.append({
            "x8": x8b[b],
            "ct8": ct8b[b],
            "wq8": _hilo_weights(Wq[sl, :].T * SC, DIM // 128),
            "wk8": _hilo_weights(Wk[sl, :].T * SC, SRC // 128),
            "wv8": _hilo_weights(wvTa * SC, SRC // 128),
            "woT": np.ascontiguousarray(Wo[:, sl].T).astype(BF),
            "rT": rt,
            "csT": csT.astype(BF),
            "idn": np.eye(128, dtype=np.float32).astype(BF),
        })
    return in_maps


def run(nc, in_maps, n_cores=8):
    res = run_bass_kernel_spmd(nc, in_maps, core_ids=list(range(n_cores)))
    return res.results


def kernel(x, context, Wq, bq, Wk, bk, Wv, bv, Wo, bo):
    B, T, _ = x.shape
    K = context.shape[1]
    x = np.asarray(x, dtype=np.float32)
    context = np.asarray(context, dtype=np.float32)
    Wq, Wk, Wv, Wo = (np.asarray(a, dtype=np.float32) for a in (Wq, Wk, Wv, Wo))
    bq, bk, bv, bo = (np.asarray(a, dtype=np.float32) for a in (bq, bk, bv, bo))

    nc = build_nc(T, K, n_cores=8)
    in_maps = make_in_maps(x, context, Wq, Wk, Wv, Wo)
    assert not bq.any() and not bk.any() and not bv.any(), "nonzero qkv bias unsupported"
    results = run(nc, in_maps)

    out = np.zeros((B, T, DIM), dtype=np.float32)
    for c in range(8):
        out[c // 4] += results[c]["y"].astype(np.float32)
    out += bo[None, None, :]
    return out


if __name__ == "__main__":
    rng = np.random.default_rng(0)
    T = K = 2048
    x = rng.standard_normal((2, T, DIM), dtype=np.float32)
    ctx = rng.standard_normal((2, K, SRC), dtype=np.float32)
    Wq = rng.standard_normal((DIM, DIM), dtype=np.float32) / 32
    Wk = rng.standard_normal((DIM, SRC), dtype=np.float32) / 34
    Wv = rng.standard_normal((DIM, SRC), dtype=np.float32) / 34
    Wo = rng.standard_normal((DIM, DIM), dtype=np.float32) / 32
    z = np.zeros(DIM, dtype=np.float32)
    got = kernel(x, ctx, Wq, z, Wk, z, Wv, z, Wo, z)

    def ref(x, ctx):
        q = x @ Wq.T
        k = ctx @ Wk.T
        v = ctx @ Wv.T
        B = x.shape[0]
        q = q.reshape(B, T, NH, HD).transpose(0, 2, 1, 3)
        k = k.reshape(B, K, NH, HD).transpose(0, 2, 1, 3)
        v = v.reshape(B, K, NH, HD).transpose(0, 2, 1, 3)
        cos, sin = _rope_tables(T, HD)

        def rot_half(t):
            t1, t2 = t[..., ::2], t[..., 1::2]
            return np.stack((-t2, t1), axis=-1).reshape(t.shape)

        q = q * cos[None, None] + rot_half(q) * sin[None, None]
        k = k * cos[None, None] + rot_half(k) * sin[None, None]
        s = np.einsum("bhtd,bhkd->bhtk", q, k) / np.sqrt(HD)
        s = np.exp(s - s.max(-1, keepdims=True))
        w = s / s.sum(-1, keepdims=True)
        o = np.einsum("bhtk,bhkd->bhtd", w, v)
        o = o.transpose(0, 2, 1, 3).reshape(B, T, DIM)
        return o @ Wo.T

    want = ref(x, ctx)
    err = np.abs(got - want).max() / np.abs(want).max()
    print("smoke relerr:", err)
